# revision 1
# baseline (speedup 1.0000x reference)
"""Trainium2 Bass kernel for nn_EnvironmentalAugmentations.

Computes, for waveform/white_noise of shape [256, 220500] fp32:
    pink  = first-order IIR of white_noise along time:
            f[0] = w[0];  f[t] = 0.99*f[t-1] + 0.01*w[t]
    mixed = waveform + 0.05 * pink
    out   = mixed / max(max|mixed|, 1.0)     (global max over all elements)

Strategy (8 NeuronCores, pure data-parallel over the 256 channels, 32/core):
  * Channels are processed in pairs: one SBUF tile [126 x 3500] holds
    channel A in partitions 0..62 and channel B in partitions 63..125,
    partition p covering 3500 consecutive samples (63 blocks per channel).
  * The IIR runs as ONE DVE `tensor_tensor_scan` per pair, in place on the
    input tile (state = a*state + w[t], zero init, per-partition).
  * Cross-partition carry: the true state entering block p is the last scan
    value of block p-1 (the a^3500 ~ 5e-16 remainder is far below fp32
    noise).  PE matmuls build the carry column in PSUM: a channel-masked
    superdiagonal shift matrix moves scan column 3499 down one partition,
    and a diagonal injector adds the t=0 initial-condition terms
    K0*w[0,0] at partitions 0 and 63 (the scan leaves column 0 equal to w).
  * The mix runs mostly on the otherwise-idle PE/ACT engines:
    carry column -> SBUF (ACT) -> PE transpose -> carry row (ACT), then PE
    accumulates  s1 = I @ waveform + carry_row (x) decay05_row  into PSUM;
    one DVE scalar_tensor_tensor computes mixed = 0.05b*g + s1 per chunk.
  * Global max: per-pair abs-max reduce -> column tile -> DVE reduce ->
    GPSIMD partition_all_reduce -> 8-core AllReduce(max) on a 4-byte DRAM
    scratch -> scale = 1/max(m,1); phase 2 rescales (DVE for resident
    pairs, ACT for spilled ones).
  * The last RES pairs stay resident in SBUF (no DRAM round-trip); earlier
    pairs spill to internal-DRAM scratch and are reloaded in phase 2.
  * DMA is split over the three descriptor engines (sync/scalar HWDGE +
    gpsimd SWDGE); every transfer is a contiguous-per-partition 1.76MB
    [126 x 14000B] block.
"""

import numpy as np

# fp32-rounded constants, mirrored from the reference
_A = float(np.float32(0.99))
_B = float(np.float32(0.01))
_NOISE = float(np.float32(0.05))

C_FULL, T_FULL = 256, 220500
N_CORES = 8
C_PER = C_FULL // N_CORES  # 32
P_USED = 126
L = T_FULL // P_USED       # 1750  (126*1750 == 220500 exactly)
RES = 7                    # channel pairs kept SBUF-resident through phase 2
K1 = 1536                  # carry-correction column cutoff (0.99^1537*40 ~ 8e-6)
PE_MIX = False             # fp32 PE matmul is 1/4-rate: PE mix not viable
QCH = 875                  # PSUM chunk width for the PE-mix (2 banks)


def _host_consts(p_used, l):
    """Constants for the pair-stacked layout [p_used, 2*l]."""
    a64, b64, n64 = _A, _B, _NOISE
    nb = p_used // 2
    lp = 2 * l
    k = np.arange(1, lp + 1, dtype=np.float64)
    decay_row = ((n64 * b64) * np.power(a64, k)).astype(np.float32)
    decay_row = np.ascontiguousarray(decay_row[None, :])      # [1, lp]
    shift = np.zeros((p_used, p_used), dtype=np.float32)
    for p in range(p_used - 1):
        if (p + 1) % nb != 0:
            shift[p, p + 1] = 1.0
    K0 = (1.0 - b64) / (b64 * a64)
    inj = np.zeros((p_used, p_used), dtype=np.float32)
    inj[0, 0] = K0
    inj[nb, nb] = K0
    ident = np.eye(p_used, dtype=np.float32)
    return decay_row, shift, inj, ident


def build_nc(c_per=C_PER, p_used=P_USED, l=L, n_cores=N_CORES, res=RES,
             pe_mix=None):
    """Build the Bacc module (per-core SPMD program)."""
    import concourse.mybir as mybir
    from concourse import bacc, bass_isa
    from concourse.tile import TileContext

    f32 = mybir.dt.float32
    Alu = mybir.AluOpType
    AxX = mybir.AxisListType.X
    t_loc = p_used * l
    assert p_used % 2 == 0
    assert c_per % 2 == 0
    nb = p_used // 2          # blocks per channel
    lp = 2 * l                # stacked row length
    n_grp = c_per // 2        # channel pairs
    res = min(res, n_grp)
    if pe_mix is None:
        pe_mix = PE_MIX
    n_spill = n_grp - res

    SC2 = _NOISE * _B         # 0.05*b, the scan-output coefficient
    # PSUM mix chunks: quarters of lp, each split into <=512 matmul pieces
    qch = min(QCH, lp)
    assert lp % qch == 0
    nq = lp // qch

    nc = bacc.Bacc(
        "TRN2", target_bir_lowering=False, debug=False, num_devices=n_cores
    )
    wave_h = nc.dram_tensor("waveform", [c_per, t_loc], f32, kind="ExternalInput")
    noise_h = nc.dram_tensor("white_noise", [c_per, t_loc], f32, kind="ExternalInput")
    decay_h = nc.dram_tensor("decayrow", [1, lp], f32, kind="ExternalInput")
    shift_h = nc.dram_tensor("shiftmat", [p_used, p_used], f32, kind="ExternalInput")
    inj_h = nc.dram_tensor("injmat", [p_used, p_used], f32, kind="ExternalInput")
    ident_h = nc.dram_tensor("identmat", [p_used, p_used], f32, kind="ExternalInput")
    out_h = nc.dram_tensor("out", [c_per, t_loc], f32, kind="ExternalOutput")

    # [(c nb), lp] views: row c*nb+p is block p of channel c (contiguous 14KB)
    wave_r = wave_h.rearrange("c (p l) -> (c p) l", p=nb)
    noise_r = noise_h.rearrange("c (p l) -> (c p) l", p=nb)
    out_r = out_h.rearrange("c (p l) -> (c p) l", p=nb)

    with TileContext(nc) as tc:
        with (
            tc.tile_pool(name="const", bufs=1) as constp,
            tc.tile_pool(name="dram", bufs=1, space="DRAM") as dramp,
        ):
            # constants (scalar queue; sync starts data loads)
            a_small = constp.tile([p_used, 1], f32, tag="a_small")
            nc.gpsimd.memset(a_small[:], _A)
            a_bc = a_small.broadcast_to([p_used, lp])
            decay_t = constp.tile([1, lp], f32, tag="decayrow")
            nc.scalar.dma_start(out=decay_t[:], in_=decay_h[:, :])
            shift_t = constp.tile([p_used, p_used], f32, tag="shift")
            nc.scalar.dma_start(out=shift_t[:], in_=shift_h[:, :])
            inj_t = constp.tile([p_used, p_used], f32, tag="injmat")
            nc.scalar.dma_start(out=inj_t[:], in_=inj_h[:, :])
            ident_t = constp.tile([p_used, p_used], f32, tag="identmat")
            nc.scalar.dma_start(out=ident_t[:], in_=ident_h[:, :])
            maxcols = constp.tile([p_used, n_grp], f32, tag="maxcols")
            k1 = min(K1, lp)
            if not pe_mix:
                decay_full = constp.tile([p_used, k1], f32, tag="decayfull")
                nc.gpsimd.partition_broadcast(
                    decay_full[:], decay_t[0:1, 0:k1], channels=p_used
                )

            with (
                tc.tile_pool(name="io1", bufs=3) as iop,
                tc.tile_pool(name="wavp", bufs=2) as wavp,
                tc.tile_pool(name="resp", bufs=1) as resp,
                tc.tile_pool(name="cps", bufs=2, space="PSUM") as cpsp,
                tc.tile_pool(name="mixps", bufs=2, space="PSUM") as mixps,
                tc.tile_pool(name="rowp", bufs=2) as rowp,
            ):
                spill_drams = []
                res_tiles = []
                w_tiles, wav_tiles = {}, {}

                def emit_loads(g):
                    rows = slice(g * p_used, (g + 1) * p_used)
                    if g >= n_spill:
                        w = resp.tile([p_used, lp], f32, tag=f"res{g}")
                    else:
                        w = iop.tile([p_used, lp], f32, tag="w")
                    nc.sync.dma_start(out=w[:], in_=noise_r[rows, :])
                    wv = wavp.tile([p_used, lp], f32, tag="wav")
                    nc.scalar.dma_start(out=wv[:], in_=wave_r[rows, :])
                    w_tiles[g], wav_tiles[g] = w, wv

                for g in range(min(3, n_grp)):
                    emit_loads(g)
                for g in range(n_grp):
                    if g + 3 < n_grp:
                        emit_loads(g + 3)
                    rows = slice(g * p_used, (g + 1) * p_used)
                    resident = g >= n_spill
                    w_t = w_tiles.pop(g)
                    wav_t = wav_tiles.pop(g)

                    # in-place per-partition IIR (column 0 stays equal to w)
                    nc.vector.tensor_tensor_scan(
                        w_t[:], a_bc, w_t[:], 0.0, Alu.mult, Alu.add
                    )
                    # carry column in PSUM
                    cp = cpsp.tile([p_used, 1], f32, tag="carry")
                    nc.tensor.matmul(
                        cp[:], shift_t[:], w_t[:, lp - 1 : lp],
                        start=True, stop=False,
                    )
                    nc.tensor.matmul(
                        cp[:], inj_t[:], w_t[:, 0:1], start=False, stop=True,
                    )
                    if pe_mix:
                        # carry column -> SBUF -> cross-partition DMA -> row
                        ccol = rowp.tile([p_used, 1], f32, tag="ccol")
                        nc.scalar.copy(ccol[:], cp[:, 0:1])
                        crow = rowp.tile([1, p_used], f32, tag="crow")
                        nc.gpsimd.dma_start(
                            out=crow[0:1, 0:p_used], in_=ccol[:, 0:1]
                        )

                        # s1 = I @ waveform + carry_row (x) decay05_row (PSUM)
                        # then mixed = 0.05b*g + s1 (DVE STT, in-place onto w)
                        for q in range(nq):
                            q0 = q * qch
                            s1q = mixps.tile([p_used, qch], f32, tag="s1q")
                            s = 0
                            while s < qch:
                                e = min(s + 512, qch)
                                nc.tensor.matmul(
                                    s1q[:, s:e], ident_t[:],
                                    wav_t[:, q0 + s : q0 + e],
                                    start=True, stop=False,
                                )
                                nc.tensor.matmul(
                                    s1q[:, s:e], crow[:],
                                    decay_t[:, q0 + s : q0 + e],
                                    start=False, stop=True,
                                )
                                s = e
                            nc.vector.scalar_tensor_tensor(
                                w_t[:, q0 : q0 + qch], w_t[:, q0 : q0 + qch],
                                float(SC2), s1q[:], Alu.mult, Alu.add,
                            )
                    else:
                        # s1 = decay05*carry + waveform (in-place onto wav;
                        # beyond column k1 the correction is < 1e-7 abs)
                        nc.vector.scalar_tensor_tensor(
                            wav_t[:, 0:k1], decay_full[:], cp[:, 0:1],
                            wav_t[:, 0:k1], Alu.mult, Alu.add,
                        )
                        # mixed = 0.05b*g + s1 (in-place onto scan tile)
                        nc.vector.scalar_tensor_tensor(
                            w_t[:], w_t[:], float(SC2), wav_t[:],
                            Alu.mult, Alu.add,
                        )
                    # per-pair abs-max
                    nc.vector.tensor_reduce(
                        maxcols[:, g : g + 1], w_t[:], AxX, Alu.max,
                        apply_absolute_value=True,
                    )
                    if resident:
                        res_tiles.append(w_t)
                    else:
                        md = dramp.tile([p_used, lp], f32, tag=f"mix{g}")
                        nc.gpsimd.dma_start(out=md[:], in_=w_t[:])
                        spill_drams.append(md)

                # ---- global max + scale ----
                allmax = constp.tile([p_used, 1], f32, tag="allmax")
                nc.vector.tensor_reduce(
                    allmax[:], maxcols[:, 0:n_grp], AxX, Alu.max
                )
                gmax = constp.tile([p_used, 1], f32, tag="gmax")
                nc.gpsimd.partition_all_reduce(
                    gmax[:], allmax[:], channels=p_used,
                    reduce_op=bass_isa.ReduceOp.max,
                )
                sc_b = constp.tile([p_used, 1], f32, tag="scb")
                if n_cores > 1:
                    cc_in = dramp.tile([1, 1], f32, tag="ccin")
                    cc_out = dramp.tile([1, 1], f32, tag="ccout")
                    nc.sync.dma_start(out=cc_in[:], in_=gmax[0:1, 0:1])
                    nc.gpsimd.collective_compute(
                        "AllReduce",
                        Alu.max,
                        replica_groups=[list(range(n_cores))],
                        ins=[cc_in[:]],
                        outs=[cc_out[:]],
                    )
                    sc_small = constp.tile([1, 1], f32, tag="scsmall")
                    nc.sync.dma_start(out=sc_small[:], in_=cc_out[:])
                    nc.gpsimd.partition_broadcast(
                        sc_b[:], sc_small[0:1, 0:1], channels=p_used
                    )
                else:
                    nc.vector.tensor_copy(sc_b[:], gmax[:])
                # scale = 1 / max(gmax, 1.0)
                nc.vector.tensor_scalar_max(sc_b[:], sc_b[:], 1.0)
                inv_t = constp.tile([p_used, 1], f32, tag="inv")
                nc.vector.reciprocal(inv_t[:], sc_b[:])

                # ---- phase 2: rescale (DVE for residents, ACT for spills).
                # Spill reloads reuse the now-idle phase-1 w/wav pool slots
                # (5 bufs of prefetch depth, no extra SBUF).
                for i, g in enumerate(range(n_spill, n_grp)):
                    rows = slice(g * p_used, (g + 1) * p_used)
                    t = res_tiles[g - n_spill]
                    nc.vector.tensor_scalar_mul(t[:], t[:], inv_t[:, 0:1])
                    dma = nc.gpsimd if i % 2 == 0 else nc.scalar
                    dma.dma_start(out=out_r[rows, :], in_=t[:])
                for g in range(n_spill):
                    rows = slice(g * p_used, (g + 1) * p_used)
                    if g % 2 == 0:
                        m_t = iop.tile([p_used, lp], f32, tag="w")
                    else:
                        m_t = wavp.tile([p_used, lp], f32, tag="wav")
                    nc.sync.dma_start(out=m_t[:], in_=spill_drams[g][:])
                    if g % 2 == 0:
                        nc.scalar.mul(m_t[:], m_t[:], inv_t[:, 0:1])
                    else:
                        nc.vector.tensor_scalar_mul(
                            m_t[:], m_t[:], inv_t[:, 0:1]
                        )
                    dma = nc.gpsimd if g % 2 == 0 else nc.scalar
                    dma.dma_start(out=out_r[rows, :], in_=m_t[:])

    nc.compile()
    return nc


_CACHE = {}
LAST_RESULTS = None


def run(waveform, white_noise, c_per=C_PER, p_used=P_USED, l=L, n_cores=N_CORES,
        **spmd_kwargs):
    """Shard inputs over n_cores, run the SPMD bass kernel, gather output."""
    global LAST_RESULTS
    from concourse.bass_utils import run_bass_kernel_spmd

    key = (c_per, p_used, l, n_cores)
    if key not in _CACHE:
        _CACHE[key] = build_nc(c_per, p_used, l, n_cores)
    nc = _CACHE[key]

    decay_row, shift, inj, ident = _host_consts(p_used, l)
    waveform = np.ascontiguousarray(waveform, dtype=np.float32)
    white_noise = np.ascontiguousarray(white_noise, dtype=np.float32)

    in_maps = []
    for i in range(n_cores):
        sl = slice(i * c_per, (i + 1) * c_per)
        in_maps.append({
            "waveform": np.ascontiguousarray(waveform[sl]),
            "white_noise": np.ascontiguousarray(white_noise[sl]),
            "decayrow": decay_row,
            "shiftmat": shift,
            "injmat": inj,
            "identmat": ident,
        })

    res = run_bass_kernel_spmd(nc, in_maps, core_ids=list(range(n_cores)),
                               **spmd_kwargs)
    LAST_RESULTS = res
    return np.concatenate([r["out"] for r in res.results], axis=0)


def kernel(waveform, white_noise):
    return run(waveform, white_noise)



# revision 3
# speedup vs baseline: 1.2464x; 1.2464x over previous
"""Trainium2 Bass kernel for nn_EnvironmentalAugmentations (v2).

Computes, for waveform/white_noise of shape [256, 220500] fp32:
    pink  = first-order IIR of white_noise along time:
            f[0] = w[0];  f[t] = 0.99*f[t-1] + 0.01*w[t]
    mixed = waveform + 0.05 * pink
    out   = mixed / max(max|mixed|, 1.0)     (global max over all elements)

Strategy (8 NeuronCores, data-parallel over channels, 32/core):
  * Channels processed in pairs: tile [126 x 3500] holds channel A in
    partitions 0..62, channel B in 63..125; partition p covers 3500
    consecutive samples (63 blocks/channel, a^3500 ~ 5e-16 between blocks).
  * v2 key change vs v1: NO DRAM spill.  The mixed signal is kept fully
    SBUF-resident in bf16 (16 pairs x 7KB/partition = 112KB/partition);
    tolerance is 2e-2 and bf16 storage costs ~3e-3, so this is safe.
    DMA traffic drops from ~117MB to the 85MB floor per core.
  * Work is spread across engines:
      ACT:  w_bf = (0.05*b) * w  (fp32->bf16, folds the mix coefficient
            into the scan input so the mix becomes a plain add);
            wav_bf = wav (fp32->bf16); boundary-column copies bf16->fp32;
            phase-2 rescale (bf16 in, fp32 out, per-partition scale).
      DVE:  tensor_tensor_scan (state = a*state + w, fp32 internal state);
            carry-correction STT on the first K1 columns; mix add
            (mix += wav_bf); per-pair abs-max reduce.
      PE:   cross-partition carry column via masked superdiagonal shift
            matmul + t=0 injection matmul into PSUM (fp32).
      gpsimd: partition_all_reduce, 4-byte AllReduce(max) across 8 cores,
            partition_broadcast, and one third of the phase-2 stores.
  * Phase 2 (after the global max): ACT rescales each resident bf16 pair
    into a recycled fp32 staging tile; stores round-robin over the
    sync/scalar/gpsimd DMA queues.
"""

import numpy as np

# fp32-rounded constants, mirrored from the reference
_A = float(np.float32(0.99))
_B = float(np.float32(0.01))
_NOISE = float(np.float32(0.05))

C_FULL, T_FULL = 256, 220500
N_CORES = 8
C_PER = C_FULL // N_CORES  # 32
P_USED = 126
L = T_FULL // P_USED       # 1750  (126*1750 == 220500 exactly)
K1 = 1536                  # carry-correction column cutoff


def _host_consts(p_used, l, k1):
    """Constants for the pair-stacked layout [p_used, 2*l]."""
    a64, b64 = _A, _B
    nb = p_used // 2
    k = np.arange(1, k1 + 1, dtype=np.float64)
    # correction applied directly to the scan output: g += carry * a^(j+1)
    decay_row = np.power(a64, k).astype(np.float32)
    decay_row = np.ascontiguousarray(decay_row[None, :])      # [1, k1]
    shift = np.zeros((p_used, p_used), dtype=np.float32)
    for p in range(p_used - 1):
        if (p + 1) % nb != 0:
            shift[p, p + 1] = 1.0
    K0 = (1.0 - b64) / (b64 * a64)
    inj = np.zeros((p_used, p_used), dtype=np.float32)
    inj[0, 0] = K0
    inj[nb, nb] = K0
    return decay_row, shift, inj


def build_nc(c_per=C_PER, p_used=P_USED, l=L, n_cores=N_CORES):
    """Build the Bacc module (per-core SPMD program)."""
    import concourse.mybir as mybir
    from concourse import bacc, bass_isa
    from concourse.tile import TileContext

    f32 = mybir.dt.float32
    bf16 = mybir.dt.bfloat16
    Alu = mybir.AluOpType
    AxX = mybir.AxisListType.X
    t_loc = p_used * l
    assert p_used % 2 == 0
    assert c_per % 2 == 0
    nb = p_used // 2          # blocks per channel
    lp = 2 * l                # stacked row length
    n_grp = c_per // 2        # channel pairs

    SC2 = _NOISE * _B         # 0.05*b, folded into the scan input

    nc = bacc.Bacc(
        "TRN2", target_bir_lowering=False, debug=False, num_devices=n_cores
    )
    wave_h = nc.dram_tensor("waveform", [c_per, t_loc], f32, kind="ExternalInput")
    noise_h = nc.dram_tensor("white_noise", [c_per, t_loc], f32, kind="ExternalInput")
    decay_h = nc.dram_tensor("decayrow", [1, K1], f32, kind="ExternalInput")
    shift_h = nc.dram_tensor("shiftmat", [p_used, p_used], f32, kind="ExternalInput")
    inj_h = nc.dram_tensor("injmat", [p_used, p_used], f32, kind="ExternalInput")
    out_h = nc.dram_tensor("out", [c_per, t_loc], f32, kind="ExternalOutput")

    # [(c nb), lp] views: row c*nb+p is block p of channel c (contiguous 14KB)
    wave_r = wave_h.rearrange("c (p l) -> (c p) l", p=nb)
    noise_r = noise_h.rearrange("c (p l) -> (c p) l", p=nb)
    out_r = out_h.rearrange("c (p l) -> (c p) l", p=nb)

    k1 = min(K1, lp)

    with TileContext(nc) as tc:
        with (
            tc.tile_pool(name="const", bufs=1) as constp,
            tc.tile_pool(name="dram", bufs=1, space="DRAM") as dramp,
        ):
            # constants (scalar queue; sync starts data loads)
            a_small = constp.tile([p_used, 1], bf16, tag="a_small")
            nc.gpsimd.memset(a_small[:], _A)
            a_bc = a_small.broadcast_to([p_used, lp])
            decay_t = constp.tile([1, k1], f32, tag="decayrow")
            nc.scalar.dma_start(out=decay_t[:], in_=decay_h[:, :])
            shift_t = constp.tile([p_used, p_used], f32, tag="shift")
            nc.scalar.dma_start(out=shift_t[:], in_=shift_h[:, :])
            inj_t = constp.tile([p_used, p_used], f32, tag="injmat")
            nc.scalar.dma_start(out=inj_t[:], in_=inj_h[:, :])
            maxcols = constp.tile([p_used, n_grp], f32, tag="maxcols")
            decay_f = constp.tile([p_used, k1], f32, tag="decayf")
            nc.gpsimd.partition_broadcast(
                decay_f[:], decay_t[0:1, 0:k1], channels=p_used
            )
            decay_bf = constp.tile([p_used, k1], bf16, tag="decaybf")
            nc.scalar.copy(decay_bf[:], decay_f[:])

            with (
                tc.tile_pool(name="resp", bufs=1) as resp,
                tc.tile_pool(name="iow", bufs=2) as iow,
                tc.tile_pool(name="iov", bufs=2) as iov,
                tc.tile_pool(name="wbf", bufs=2) as wbfp,
                tc.tile_pool(name="colp", bufs=3) as colp,
                tc.tile_pool(name="cps", bufs=2, space="PSUM") as cpsp,
            ):
                mix_tiles = [
                    resp.tile([p_used, lp], bf16, tag=f"mix{g}",
                              name=f"mix{g}")
                    for g in range(n_grp)
                ]
                w_tiles, wav_tiles, wavbf_tiles, carry_ps = {}, {}, {}, {}

                def emit_loads(g):
                    rows = slice(g * p_used, (g + 1) * p_used)
                    w = iow.tile([p_used, lp], f32, tag="w")
                    nc.sync.dma_start(out=w[:], in_=noise_r[rows, :])
                    wv = iov.tile([p_used, lp], f32, tag="wav")
                    nc.scalar.dma_start(out=wv[:], in_=wave_r[rows, :])
                    w_tiles[g], wav_tiles[g] = w, wv

                DEPTH = 2
                for g in range(min(DEPTH, n_grp)):
                    emit_loads(g)

                for g in range(n_grp + 1):
                    if g < n_grp:
                        if g + DEPTH < n_grp:
                            emit_loads(g + DEPTH)
                        mix = mix_tiles[g]
                        w_t = w_tiles.pop(g)
                        wav_t = wav_tiles.pop(g)
                        # ACT: fold 0.05*b into the scan input, fp32 -> bf16
                        nc.scalar.mul(mix[:], w_t[:], SC2)
                        # ACT: waveform fp32 -> bf16
                        wav_bf = wbfp.tile([p_used, lp], bf16, tag="wavbf")
                        nc.scalar.copy(wav_bf[:], wav_t[:])
                        wavbf_tiles[g] = wav_bf
                        # DVE: in-place per-partition IIR scan
                        nc.vector.tensor_tensor_scan(
                            mix[:], a_bc, mix[:], 0.0, Alu.mult, Alu.add
                        )
                    if g >= 1:
                        gp = g - 1
                        mixp = mix_tiles[gp]
                        # ACT: boundary columns bf16 -> fp32 for the PE carry
                        col_last = colp.tile([p_used, 1], f32, tag="cl")
                        nc.scalar.copy(col_last[:], mixp[:, lp - 1 : lp])
                        col_first = colp.tile([p_used, 1], f32, tag="cf")
                        nc.scalar.copy(col_first[:], mixp[:, 0:1])
                        # PE: carry column in PSUM
                        cp = cpsp.tile([p_used, 1], f32, tag="carry")
                        nc.tensor.matmul(
                            cp[:], shift_t[:], col_last[:], start=True, stop=False
                        )
                        nc.tensor.matmul(
                            cp[:], inj_t[:], col_first[:], start=False, stop=True
                        )
                        # DVE: carry correction on the first k1 columns
                        nc.vector.scalar_tensor_tensor(
                            mixp[:, 0:k1], decay_bf[:], cp[:, 0:1],
                            mixp[:, 0:k1], Alu.mult, Alu.add,
                        )
                        # DVE: mix += waveform (bf16 add)
                        nc.vector.tensor_tensor(
                            mixp[:], mixp[:], wavbf_tiles.pop(gp)[:], Alu.add
                        )
                        # DVE: per-pair abs-max
                        nc.vector.tensor_reduce(
                            maxcols[:, gp : gp + 1], mixp[:], AxX, Alu.max,
                            apply_absolute_value=True,
                        )

                # ---- global max + scale ----
                allmax = constp.tile([p_used, 1], f32, tag="allmax")
                nc.vector.tensor_reduce(
                    allmax[:], maxcols[:, 0:n_grp], AxX, Alu.max
                )
                gmax = constp.tile([p_used, 1], f32, tag="gmax")
                nc.gpsimd.partition_all_reduce(
                    gmax[:], allmax[:], channels=p_used,
                    reduce_op=bass_isa.ReduceOp.max,
                )
                sc_b = constp.tile([p_used, 1], f32, tag="scb")
                if n_cores > 1:
                    cc_in = dramp.tile([1, 1], f32, tag="ccin")
                    cc_out = dramp.tile([1, 1], f32, tag="ccout")
                    nc.sync.dma_start(out=cc_in[:], in_=gmax[0:1, 0:1])
                    nc.gpsimd.collective_compute(
                        "AllReduce",
                        Alu.max,
                        replica_groups=[list(range(n_cores))],
                        ins=[cc_in[:]],
                        outs=[cc_out[:]],
                    )
                    sc_small = constp.tile([1, 1], f32, tag="scsmall")
                    nc.sync.dma_start(out=sc_small[:], in_=cc_out[:])
                    nc.gpsimd.partition_broadcast(
                        sc_b[:], sc_small[0:1, 0:1], channels=p_used
                    )
                else:
                    nc.vector.tensor_copy(sc_b[:], gmax[:])
                # scale = 1 / max(gmax, 1.0)
                nc.vector.tensor_scalar_max(sc_b[:], sc_b[:], 1.0)
                inv_t = constp.tile([p_used, 1], f32, tag="inv")
                nc.vector.reciprocal(inv_t[:], sc_b[:])

                # ---- phase 2: ACT rescale bf16 -> fp32, store on 3 queues.
                # Staging reuses the now-idle load pools (4 fp32 bufs).
                dmas = [nc.sync, nc.scalar, nc.gpsimd]
                for g in range(n_grp):
                    rows = slice(g * p_used, (g + 1) * p_used)
                    if g % 2 == 0:
                        st = iow.tile([p_used, lp], f32, tag="w")
                    else:
                        st = iov.tile([p_used, lp], f32, tag="wav")
                    nc.scalar.mul(st[:], mix_tiles[g][:], inv_t[:, 0:1])
                    dmas[g % 3].dma_start(out=out_r[rows, :], in_=st[:])

    nc.compile()
    return nc


_CACHE = {}
LAST_RESULTS = None


def run(waveform, white_noise, c_per=C_PER, p_used=P_USED, l=L, n_cores=N_CORES,
        **spmd_kwargs):
    """Shard inputs over n_cores, run the SPMD bass kernel, gather output."""
    global LAST_RESULTS
    from concourse.bass_utils import run_bass_kernel_spmd

    key = (c_per, p_used, l, n_cores)
    if key not in _CACHE:
        _CACHE[key] = build_nc(c_per, p_used, l, n_cores)
    nc = _CACHE[key]

    decay_row, shift, inj = _host_consts(p_used, l, K1)
    waveform = np.ascontiguousarray(waveform, dtype=np.float32)
    white_noise = np.ascontiguousarray(white_noise, dtype=np.float32)

    in_maps = []
    for i in range(n_cores):
        sl = slice(i * c_per, (i + 1) * c_per)
        in_maps.append({
            "waveform": np.ascontiguousarray(waveform[sl]),
            "white_noise": np.ascontiguousarray(white_noise[sl]),
            "decayrow": decay_row,
            "shiftmat": shift,
            "injmat": inj,
        })

    res = run_bass_kernel_spmd(nc, in_maps, core_ids=list(range(n_cores)),
                               **spmd_kwargs)
    LAST_RESULTS = res
    return np.concatenate([r["out"] for r in res.results], axis=0)


def kernel(waveform, white_noise):
    return run(waveform, white_noise)
